# revision 8
# baseline (speedup 1.0000x reference)
"""Trainium2 Bass kernel for 16-head causal multi-head attention.

Contract: kernel(**inputs) takes the FULL unsharded inputs of
nn_MultiHeadAttention (q/k/v [2, 2048, 1024] f32, mask [1,1,2048,2048] bool,
w_q/w_k/w_v/w_o [1024, 1024] f32, biases [1024] f32) and returns the full
[2, 2048, 1024] f32 output.

Sharding (Megatron-style, hardcoded for 8 cores):
  - w_q/w_k/w_v column-parallel: core c owns heads 2c, 2c+1 (128 output dims).
  - Attention per core: its 2 heads x both batches.
  - w_o row-parallel: core c contracts its 128 dims -> partial [1024, 4096]
    output (transposed), host sums the 8 partials and adds b_o.

Device layout notes:
  - All activations live transposed ([feature, token]) because the PE array
    contracts over the partition dim.  Host pre-transposes q/k/v once (f16).
  - Scores are computed transposed (S^T[k, q]) so that exp'd probabilities
    are directly the moving operand of the attention*V matmul; row-sums come
    free via a ones-column appended to V.  Softmax max-subtraction is skipped
    (scores are ~N(0,1) after the 1/sqrt(dk) scale; exp cannot overflow).
  - The causal mask is applied multiplicatively post-exp on 128x128 diagonal
    blocks only; fully-masked blocks are skipped.  The block structure is
    derived from the mask input at compile time, so any block-structured mask
    works, not just causal.
"""

import sys

import numpy as np

_REPO = "/opt/trn_rl_repo"
if _REPO not in sys.path:
    sys.path.insert(0, _REPO)

B, S, D = 2, 2048, 1024
H, DK = 16, 64
T = B * S  # 4096 tokens total
NCORES = 8
D2 = 128  # head dims per core (2 heads x 64)
NJ = D // 128  # contraction chunks for projections
NKC = S // 128  # k-chunks per batch
NQB = S // 512  # q-blocks of 512 per batch
NCHUNK = T // 128  # 32 global k-chunks
SCALE = 1.0 / float(np.sqrt(DK))

_prog_cache: dict = {}


def _analyze_mask(mask: np.ndarray):
    """Classify each 128x128 (k-chunk, q-sub) block of the mask.

    Returns (states, patterns): states[(kc, qs)] = ("full"|"empty"|"partial",
    pattern_id); patterns = list of [128,128] f16 multiplicative masks laid
    out [k, q] (transposed score block orientation).
    """
    m = np.asarray(mask).reshape(S, S)  # mask[q, k], True = masked out
    states = {}
    patterns = []
    ids = {}
    for kc in range(NKC):
        for qs in range(NKC):
            blk = m[qs * 128:(qs + 1) * 128, kc * 128:(kc + 1) * 128]
            if not blk.any():
                states[(kc, qs)] = ("full", 0)
            elif blk.all():
                states[(kc, qs)] = ("empty", 0)
            else:
                pat = (~blk.T).astype(np.float16)  # [k, q] keep-multiplier
                key = pat.tobytes()
                if key not in ids:
                    ids[key] = len(patterns)
                    patterns.append(pat)
                states[(kc, qs)] = ("partial", ids[key])
    return states, patterns


def fixed_tc_class():
    """TileContext subclass working around walrus sync-wait capacity."""
    import bass_rust
    import concourse.mybir as mybir
    import concourse.tile as tile

    class _TC(tile.TileContext):
        # walrus in this container has small per-instruction sync-wait
        # capacities (1 for NO_STRUCT drains/nops, ~2 for structured ops).
        # After scheduling, split excess waits onto preceding same-engine
        # NOPs (each carrying a single wait), which is semantically
        # identical: the engine blocks in program order either way.
        WAIT_KEEP_DEFAULT = 1

        def _split_excess_waits(self):
            uid = [0]

            def mk_nop(engine, wait):
                uid[0] += 1
                nop = mybir.InstNoOp(name=f"I-waitsplit-{uid[0]}", ins=[], outs=[])
                nop.engine = engine
                nop.sync_info = mybir.SyncInfo(on_wait=[wait], on_update=[])
                return nop

            for f in self.nc.m.functions:
                for blk in f.blocks:
                    out = []
                    for inst in blk.instructions:
                        si = inst.sync_info
                        keep = (1 if isinstance(inst, (mybir.InstDrain, mybir.InstNoOp))
                                else self.WAIT_KEEP_DEFAULT)
                        if si is not None and si.on_wait and len(si.on_wait) > keep:
                            waits = list(si.on_wait)
                            si.on_wait = waits[:keep]
                            for w in waits[keep:]:
                                out.append(mk_nop(inst.engine, w))
                        out.append(inst)
                    blk.instructions[:] = out

        def _drain_and_barrier(self, tick_clock, wait_clock):
            drain_inst = self.nc.sync.drain()
            wait_clock.add_sem_waits(
                drain_inst.ins,
                bass_rust.ScopedClock({None: tick_clock.global_clock}),
            )
            self.nc.all_engine_barrier()
            assert self.sems is not None
            popped = self.nc._tile_sem_poison_stack.pop()
            assert popped is self._sem_poison
            self.nc.clear_and_free_semaphores(list(self.sems.allocated().values()))
            self.nc.all_engine_barrier()
            self._split_excess_waits()

    return _TC


def _build(states, npat):
    """Emit the Bass/Tile program.  Returns the finalized Bass module."""
    import concourse.bass as bass
    import concourse.mybir as mybir
    from contextlib import ExitStack

    f16, f32 = mybir.dt.float16, mybir.dt.float32
    AF = mybir.ActivationFunctionType
    _TC = fixed_tc_class()

    nc = bass.Bass("TRN2", target_bir_lowering=False, debug=False, num_devices=1)

    dr = {}
    for nm, shape, dt in [
        ("xq", [D, T], f16), ("xk", [D, T], f16), ("xv", [D, T], f16),
        ("wq", [D, D2], f16), ("wk", [D, D2], f16), ("wv", [D, D2], f16),
        ("wo", [D2, D], f16),
        ("bq", [D2, 1], f32), ("bk", [D2, 1], f32), ("bv", [D2, 1], f32),
        ("ident", [128, 128], f16),
        ("pats", [max(npat, 1) * 128, 128], f16),
    ]:
        dr[nm] = nc.dram_tensor(nm, shape, dt, kind="ExternalInput").ap()
    ot = nc.dram_tensor("ot", [D, T], f16, kind="ExternalOutput").ap()

    # kc lists per (b is irrelevant: same mask) q-block
    kc_lists = []
    for qb in range(NQB):
        qss = range(qb * 4, qb * 4 + 4)
        lst = [kc for kc in range(NKC)
               if any(states[(kc, qs)][0] != "empty" for qs in qss)]
        kc_lists.append(lst)

    with _TC(nc) as tc, ExitStack() as ctx:
        persist = ctx.enter_context(tc.tile_pool(name="persist", bufs=1))
        xin_pool = ctx.enter_context(tc.tile_pool(name="xin", bufs=9))
        pt_pool = ctx.enter_context(tc.tile_pool(name="pt", bufs=6))
        osb_pool = ctx.enter_context(tc.tile_pool(name="osb", bufs=4))
        bc_pool = ctx.enter_context(tc.tile_pool(name="bc", bufs=2))
        rc_pool = ctx.enter_context(tc.tile_pool(name="rc", bufs=2))
        ppool = ctx.enter_context(tc.tile_pool(name="pp", bufs=2, space="PSUM"))
        trpool = ctx.enter_context(tc.tile_pool(name="tr", bufs=1, space="PSUM"))
        spool = ctx.enter_context(tc.tile_pool(name="sp", bufs=3, space="PSUM"))
        avpool = ctx.enter_context(tc.tile_pool(name="av", bufs=2, space="PSUM"))

        # ---- persistent SBUF tensors ----
        QT = persist.tile([128, T], f16, tag="QT")
        KT = persist.tile([128, T], f16, tag="KT")
        VT = persist.tile([128, T], f16, tag="VT")
        ANT = persist.tile([128, T], f16, tag="ANT")  # normalized attn^T
        VA = persist.tile([128, NCHUNK * 65], f16, tag="VA")  # [V_A | 1] chunks
        VB = persist.tile([128, NCHUNK * 65], f16, tag="VB")
        WQ = persist.tile([128, D], f16, tag="WQ")
        WK = persist.tile([128, D], f16, tag="WK")
        WV = persist.tile([128, D], f16, tag="WV")
        WO = persist.tile([128, D], f16, tag="WO")
        BQ = persist.tile([128, 1], f32, tag="BQ")
        BK = persist.tile([128, 1], f32, tag="BK")
        BV = persist.tile([128, 1], f32, tag="BV")
        IDT = persist.tile([128, 128], f16, tag="IDT")
        MSK = persist.tile([128, max(npat, 1) * 128], f16, tag="MSK")

        # ---- constants / weights load ----
        for j in range(NJ):
            nc.sync.dma_start(WQ[:, j * 128:(j + 1) * 128], dr["wq"][j * 128:(j + 1) * 128, :])
            nc.sync.dma_start(WK[:, j * 128:(j + 1) * 128], dr["wk"][j * 128:(j + 1) * 128, :])
            nc.sync.dma_start(WV[:, j * 128:(j + 1) * 128], dr["wv"][j * 128:(j + 1) * 128, :])
        nc.sync.dma_start(WO[:, :], dr["wo"][:, :])
        nc.sync.dma_start(BQ[:, :], dr["bq"][:, :])
        nc.sync.dma_start(BK[:, :], dr["bk"][:, :])
        nc.sync.dma_start(BV[:, :], dr["bv"][:, :])
        nc.sync.dma_start(IDT[:, :], dr["ident"][:, :])
        for p in range(max(npat, 1)):
            nc.sync.dma_start(MSK[:, p * 128:(p + 1) * 128], dr["pats"][p * 128:(p + 1) * 128, :])
        nc.vector.memset(VA[:, :], 1.0)
        nc.vector.memset(VB[:, :], 1.0)

        def projection(b, W, BIAS, XDR, XT):
            """Q/K/V projection for batch b: XT[:, b*S:(b+1)*S] = W.T@x^T + bias."""
            xins = []
            for j in range(NJ):
                xin = xin_pool.tile([128, S], f16, tag="xin")
                nc.sync.dma_start(xin[:, :], XDR[j * 128:(j + 1) * 128, b * S:(b + 1) * S])
                xins.append(xin)
            for fb in range(S // 512):
                ps = ppool.tile([128, 512], f32, tag="pp")
                for j in range(NJ):
                    nc.tensor.matmul(
                        ps[:, :],
                        lhsT=W[:, j * 128:(j + 1) * 128],
                        rhs=xins[j][:, fb * 512:(fb + 1) * 512],
                        start=(j == 0), stop=(j == NJ - 1),
                    )
                nc.scalar.activation(
                    XT[:, b * S + fb * 512: b * S + (fb + 1) * 512],
                    ps[:, :], AF.Identity, bias=BIAS[:, 0:1],
                )

        for b in range(B):
            base = b * S
            projection(b, WQ, BQ, dr["xq"], QT)
            projection(b, WK, BK, dr["xk"], KT)
            projection(b, WV, BV, dr["xv"], VT)

            # ---- V transform: VT chunks -> natural layout + ones column ----
            for t in range(NKC):
                g = b * NKC + t
                trp = trpool.tile([128, 128], f16, tag="tr")
                nc.tensor.transpose(trp[:, :], VT[:, g * 128:(g + 1) * 128], IDT[:, :])
                nc.vector.tensor_copy(VA[:, g * 65:g * 65 + 64], trp[:, 0:64])
                nc.vector.tensor_copy(VB[:, g * 65:g * 65 + 64], trp[:, 64:128])

            # ---- attention ----
            for qb in range(NQB):
                qlo = base + qb * 512  # global token offset of this q-block
                kcl = kc_lists[qb]
                av_a = avpool.tile([128, 512], f32, tag="av")
                av_b = avpool.tile([128, 512], f32, tag="av")
                for i, kc in enumerate(kcl):
                    g = b * NKC + kc
                    ksl = slice(base + kc * 128, base + (kc + 1) * 128)
                    st = [states[(kc, qs)] for qs in range(qb * 4, qb * 4 + 4)]
                    all_full = all(s[0] == "full" for s in st)
                    pa = pt_pool.tile([128, 512], f16, tag="pt")
                    pb = pt_pool.tile([128, 512], f16, tag="pt")
                    if all_full:
                        sa = spool.tile([128, 512], f32, tag="sp")
                        sb = spool.tile([128, 512], f32, tag="sp")
                        nc.tensor.matmul(sa[:, :], lhsT=KT[0:64, ksl], rhs=QT[0:64, qlo:qlo + 512], start=True, stop=True)
                        nc.tensor.matmul(sb[:, :], lhsT=KT[64:128, ksl], rhs=QT[64:128, qlo:qlo + 512], start=True, stop=True)
                        nc.scalar.activation(pa[:, :], sa[:, :], AF.Exp, scale=SCALE)
                        nc.scalar.activation(pb[:, :], sb[:, :], AF.Exp, scale=SCALE)
                    else:
                        sa = spool.tile([128, 512], f32, tag="sp")
                        sb = spool.tile([128, 512], f32, tag="sp")
                        for u in range(4):
                            kind, pid = st[u]
                            usl = slice(u * 128, (u + 1) * 128)
                            if kind == "empty":
                                nc.vector.memset(pa[:, usl], 0.0)
                                nc.vector.memset(pb[:, usl], 0.0)
                                continue
                            qsl = slice(qlo + u * 128, qlo + (u + 1) * 128)
                            nc.tensor.matmul(sa[:, usl], lhsT=KT[0:64, ksl], rhs=QT[0:64, qsl], start=True, stop=True)
                            nc.tensor.matmul(sb[:, usl], lhsT=KT[64:128, ksl], rhs=QT[64:128, qsl], start=True, stop=True)
                            nc.scalar.activation(pa[:, usl], sa[:, usl], AF.Exp, scale=SCALE)
                            nc.scalar.activation(pb[:, usl], sb[:, usl], AF.Exp, scale=SCALE)
                            if kind == "partial":
                                msl = slice(pid * 128, (pid + 1) * 128)
                                nc.vector.tensor_mul(pa[:, usl], pa[:, usl], MSK[:, msl])
                                nc.vector.tensor_mul(pb[:, usl], pb[:, usl], MSK[:, msl])
                    last = (i == len(kcl) - 1)
                    nc.tensor.matmul(av_a[0:65, :], lhsT=VA[:, g * 65:(g + 1) * 65], rhs=pa[:, :], start=(i == 0), stop=last)
                    nc.tensor.matmul(av_b[0:65, :], lhsT=VB[:, g * 65:(g + 1) * 65], rhs=pb[:, :], start=(i == 0), stop=last)

                # ---- normalize: divide by row-sums (row 64 of av psum) ----
                rc = rc_pool.tile([33, 512], f32, tag="rc")
                nc.vector.reciprocal(rc[0:1, :], av_a[64:65, :])
                nc.vector.reciprocal(rc[32:33, :], av_b[64:65, :])
                bc = bc_pool.tile([128, 512], f32, tag="bc")
                nc.sync.dma_start(bc[0:64, :], rc[0:1, None, :].broadcast_to((1, 64, 512)))
                nc.sync.dma_start(bc[64:128, :], rc[32:33, None, :].broadcast_to((1, 64, 512)))
                nc.vector.tensor_mul(ANT[0:64, qlo:qlo + 512], av_a[0:64, :], bc[0:64, :])
                nc.vector.tensor_mul(ANT[64:128, qlo:qlo + 512], av_b[0:64, :], bc[64:128, :])

            # ---- output projection (row-parallel partial) ----
            for dm in range(NJ):
                for sb_i in range(S // 512):
                    ps = ppool.tile([128, 512], f32, tag="pp")
                    nc.tensor.matmul(
                        ps[:, :], lhsT=WO[:, dm * 128:(dm + 1) * 128],
                        rhs=ANT[:, base + sb_i * 512: base + (sb_i + 1) * 512],
                        start=True, stop=True,
                    )
                    ob = osb_pool.tile([128, 512], f16, tag="osb")
                    nc.scalar.copy(ob[:, :], ps[:, :])
                    nc.sync.dma_start(
                        ot[dm * 128:(dm + 1) * 128, base + sb_i * 512: base + (sb_i + 1) * 512],
                        ob[:, :],
                    )
    return nc


def _prep_inputs(q, k, v, mask, w_q, b_q, w_k, b_k, w_v, b_v, w_o, b_o):
    """Host-side sharding/layout prep.  Returns (in_maps, mask_key, host)."""
    states, patterns = _analyze_mask(mask)
    qT = np.ascontiguousarray(np.asarray(q, np.float32).reshape(T, D).T.astype(np.float16))
    kT = np.ascontiguousarray(np.asarray(k, np.float32).reshape(T, D).T.astype(np.float16))
    vT = np.ascontiguousarray(np.asarray(v, np.float32).reshape(T, D).T.astype(np.float16))
    ident = np.eye(128, dtype=np.float16)
    npat = len(patterns)
    pats = (np.concatenate(patterns, axis=0) if npat
            else np.zeros((128, 128), np.float16))
    w_q = np.asarray(w_q, np.float32)
    w_k = np.asarray(w_k, np.float32)
    w_v = np.asarray(w_v, np.float32)
    w_o = np.asarray(w_o, np.float32)
    in_maps = []
    for c in range(NCORES):
        hb = c * D2
        in_maps.append({
            "xq": qT, "xk": kT, "xv": vT,
            "wq": np.ascontiguousarray(w_q[hb:hb + D2, :].T.astype(np.float16)),
            "wk": np.ascontiguousarray(w_k[hb:hb + D2, :].T.astype(np.float16)),
            "wv": np.ascontiguousarray(w_v[hb:hb + D2, :].T.astype(np.float16)),
            "wo": np.ascontiguousarray(w_o[:, hb:hb + D2].T.astype(np.float16)),
            "bq": np.asarray(b_q, np.float32)[hb:hb + D2].reshape(D2, 1),
            "bk": np.asarray(b_k, np.float32)[hb:hb + D2].reshape(D2, 1),
            "bv": np.asarray(b_v, np.float32)[hb:hb + D2].reshape(D2, 1),
            "ident": ident,
            "pats": np.ascontiguousarray(pats),
        })
    mask_key = (tuple(sorted((k_, v_) for k_, v_ in states.items())), npat)
    return in_maps, (states, npat, mask_key), np.asarray(b_o, np.float32)


def get_program(states, npat, mask_key):
    if mask_key not in _prog_cache:
        _prog_cache[mask_key] = _build(states, npat)
    return _prog_cache[mask_key]


def _reduce_output(results, b_o):
    acc = np.zeros((D, T), np.float32)
    for r in results:
        acc += r["ot"].astype(np.float32)
    out = acc.T + b_o[None, :]
    return np.ascontiguousarray(out.reshape(B, S, D).astype(np.float32))


def kernel(q, k, v, mask, w_q, b_q, w_k, b_k, w_v, b_v, w_o, b_o):
    from concourse import bass_utils

    in_maps, (states, npat, mask_key), b_o_f = _prep_inputs(
        q, k, v, mask, w_q, b_q, w_k, b_k, w_v, b_v, w_o, b_o)
    nc = get_program(states, npat, mask_key)
    res = bass_utils.run_bass_kernel_spmd(nc, in_maps, core_ids=list(range(NCORES)))
    return _reduce_output(res.results, b_o_f)
